# revision 1
# baseline (speedup 1.0000x reference)
"""BiSSM (bidirectional Mamba block) Trainium2 kernel.

Contract: kernel(**inputs) takes the FULL unsharded inputs of
nn_BiSSMBlock (see setup_inputs) and returns the full (2, 1024, 1024)
output.  Internally shards across 8 NeuronCores as
(batch 2) x (direction 2) x (d_inner half 2); each core runs an identical
Bass program on host-sliced data, with a pairwise AllReduce between
half-cores for the x_dbl projection.  Host folds Wout into proj_W
(per direction) and sums/flips partials.

Self-contained: only needs the concourse/bass toolchain at
/opt/trn_rl_repo and 8 visible neuron cores.
"""
import sys
sys.path.insert(0, "/opt/trn_rl_repo")
import numpy as np

import concourse.bass as bass
import concourse.bacc as bacc
import concourse.mybir as mybir
import concourse.tile as tile

F32 = mybir.dt.float32
F32R = mybir.dt.float32r
BF16 = mybir.dt.bfloat16
OP = mybir.AluOpType
AF = mybir.ActivationFunctionType

L = 1024          # sequence length
DM = 1024         # d_model
DH = 1024         # d_inner half per core
NG = 8            # channel segments (DH/128)
NST = 16          # d_state
NT = 2            # time chunks of 512 for matmul moving dim
TN = 512

N_CORES = 8
CC_GROUPS = [[0, 1], [2, 3], [4, 5], [6, 7]]


def _build(structured_a, g_blk=4, scan_dt=BF16, xin_dt=BF16, n_cores=N_CORES):
    nc = bacc.Bacc("TRN2", target_bir_lowering=False, debug=False, num_devices=n_cores)
    NB = NG // g_blk
    FB = g_blk * L

    xT = nc.declare_dram_parameter("xT", [DM, L], F32, isOutput=False)
    w_in = nc.declare_dram_parameter("w_in", [DM, 2 * DH], F32, isOutput=False)
    wx = nc.declare_dram_parameter("wx", [DH, 96], F32, isOutput=False)
    wdt = nc.declare_dram_parameter("wdt", [64, DH], F32, isOutput=False)
    wco = nc.declare_dram_parameter("wco", [DH, DM], F32, isOutput=False)
    convw = nc.declare_dram_parameter("convw", [128, NG, 4], F32, isOutput=False)
    convb = nc.declare_dram_parameter("convb", [128, NG], F32, isOutput=False)
    binz = nc.declare_dram_parameter("binz", [128, NG], F32, isOutput=False)
    bdt = nc.declare_dram_parameter("bdt", [128, NG], F32, isOutput=False)
    A_ = nc.declare_dram_parameter("A_", [128, NG, NST], F32, isOutput=False)
    dsk = nc.declare_dram_parameter("dsk", [128, NG], F32, isOutput=False)
    outp = nc.declare_dram_parameter("outp", [2, DM, L], F32, isOutput=True)

    cc_in = nc.dram_tensor("cc_in", [96, L], F32)
    cc_out = nc.dram_tensor("cc_out", [96, L], F32)
    cc_bf = nc.dram_tensor("cc_bf", [32, L], BF16)

    with tile.TileContext(nc) as tc:
        consts_cm = tc.tile_pool(name="consts", bufs=1)
        consts = consts_cm.__enter__()
        cw = consts.tile([128, NG, 4], F32)
        nc.sync.dma_start(out=cw[:], in_=convw[:])
        cb = consts.tile([128, NG], F32)
        nc.sync.dma_start(out=cb[:], in_=convb[:])
        bz = consts.tile([128, NG], F32)
        nc.sync.dma_start(out=bz[:], in_=binz[:])
        bd = consts.tile([128, NG], F32)
        nc.sync.dma_start(out=bd[:], in_=bdt[:])
        At = consts.tile([128, NG, NST], F32)
        nc.sync.dma_start(out=At[:], in_=A_[:])
        dk = consts.tile([128, NG], F32)
        nc.sync.dma_start(out=dk[:], in_=dsk[:])

        yacp_cm = tc.tile_pool(name="yacp", bufs=1)
        yacp = yacp_cm.__enter__()
        yac = yacp.tile([128, NG, L], F32, tag="yac")

        szgp_cm = tc.tile_pool(name="szgp", bufs=1)
        szgp = szgp_cm.__enter__()
        szg = szgp.tile([128, NG, L], scan_dt, tag="szg")

        poolD_cm = tc.tile_pool(name="poolD", bufs=1)
        poolD = poolD_cm.__enter__()
        delta = poolD.tile([128, NG, L], F32, tag="delta")
        du = poolD.tile([128, NG, L], scan_dt, tag="du")

        poolU_cm = tc.tile_pool(name="poolU", bufs=1)
        poolU = poolU_cm.__enter__()
        xin = poolU.tile([128, NG, L + 3], xin_dt, tag="xin")
        u3 = poolU.tile([128, NG, L], BF16, tag="u3")
        halo = bass.AP(tensor=xin.tensor, offset=xin.offset,
                       ap=[[xin.ap[0][0], 128], [L + 3, NG], [1, 3]])
        nc.vector.memset(halo, 0.0)

        w_in_r = w_in.ap().rearrange("(k p) m -> p k m", p=128).bitcast(F32R)

        # ---------------- Phase A1: in_proj (xin half; z deferred past collective) ----------------
        with tc.tile_pool(name="poolW", bufs=1) as poolW, \
             tc.tile_pool(name="wchunk", bufs=2) as wchunk, \
             tc.tile_pool(name="mm_ps", bufs=4, space="PSUM") as mm_ps:
            xts = poolW.tile([128, 8, L], F32R, tag="xts")
            nc.sync.dma_start(out=xts[:], in_=xT.ap().rearrange("(k p) t -> p k t", p=128).bitcast(F32R))
            for m in range(8):
                wi = wchunk.tile([128, 8, 128], F32R, tag="wi")
                nc.sync.dma_start(out=wi[:], in_=w_in_r[:, :, m * 128:(m + 1) * 128])
                for tn in range(NT):
                    ps = mm_ps.tile([128, TN], F32, tag="ps")
                    for k in range(8):
                        nc.tensor.matmul(ps[:], wi[:, k, :],
                                         xts[:, k, tn * TN:(tn + 1) * TN],
                                         start=(k == 0), stop=(k == 7))
                    nc.scalar.copy(out=xin[:, m, 3 + tn * TN: 3 + (tn + 1) * TN], in_=ps[:])

            # conv (fp32 scratch per segment) + silu -> u (bf16)
            for g in range(NG):
                scr = wchunk.tile([128, L], F32, tag="scr")
                nc.vector.tensor_scalar_mul(out=scr[:], in0=xin[:, g, 3:3 + L],
                                            scalar1=cw[:, g, 3:4])
                for k in range(3):
                    nc.vector.scalar_tensor_tensor(
                        out=scr[:], in0=xin[:, g, k:k + L],
                        scalar=cw[:, g, k:k + 1], in1=scr[:],
                        op0=OP.mult, op1=OP.add)
                nc.scalar.activation(out=u3[:, g, :], in_=scr[:], func=AF.Silu,
                                     bias=cb[:, g:g + 1], scale=1.0)
                nc.vector.tensor_scalar_mul(out=yac[:, g, :], in0=u3[:, g, :], scalar1=dk[:, g:g + 1])

            # ------------ Phase A2: Wx partial + AllReduce + z + delta ------------
            wx_ps = mm_ps
            wxp = poolW
            wxs = wxp.tile([128, 8, 96], BF16, tag="wxs")
            nc.gpsimd.dma_start(out=wxs[:], in_=wx.ap().rearrange("(k p) m -> p k m", p=128))
            xdb = wxp.tile([96, L], F32, tag="xdb")
            for tn in range(NT):
                ps96 = wx_ps.tile([96, TN], F32, tag="ps96")
                for k in range(8):
                    nc.tensor.matmul(ps96[:], wxs[:, k, :],
                                     u3[:, k, tn * TN:(tn + 1) * TN],
                                     start=(k == 0), stop=(k == 7))
                nc.vector.tensor_copy(out=xdb[:, tn * TN:(tn + 1) * TN], in_=ps96[:])
            nc.sync.dma_start(out=cc_in[:], in_=xdb[:])
            nc.gpsimd.collective_compute("AllReduce", OP.add, replica_groups=CC_GROUPS,
                                         ins=[cc_in[:]], outs=[cc_out[:]])
            nc.gpsimd.dma_start(out=cc_bf[:], in_=cc_out[64:96, :])

            # z half of in_proj: emitted after the collective so its PE/ACT work
            # fills the AllReduce wait window
            for m in range(8):
                wi = wchunk.tile([128, 8, 128], F32R, tag="wi")
                nc.sync.dma_start(out=wi[:], in_=w_in_r[:, :, DH + m * 128: DH + (m + 1) * 128])
                for tn in range(NT):
                    ps = mm_ps.tile([128, TN], F32, tag="ps")
                    for k in range(8):
                        nc.tensor.matmul(ps[:], wi[:, k, :],
                                         xts[:, k, tn * TN:(tn + 1) * TN],
                                         start=(k == 0), stop=(k == 7))
                    nc.scalar.activation(out=szg[:, m, tn * TN:(tn + 1) * TN],
                                         in_=ps[:], func=AF.Silu,
                                         bias=bz[:, m:m + 1], scale=1.0)

            dts = wxp.tile([64, L], F32R, tag="dts")
            nc.sync.dma_start(out=dts[:], in_=cc_out[0:64, :].bitcast(F32R))
            wds = wxp.tile([64, NG, 128], F32R, tag="wds")
            nc.sync.dma_start(out=wds[:], in_=wdt.ap().rearrange("k (g p) -> k g p", p=128).bitcast(F32R))
            for g in range(NG):
                for tn in range(NT):
                    psd = wx_ps.tile([128, TN], F32, tag="ps")
                    nc.tensor.matmul(psd[:], wds[:, g, :], dts[:, tn * TN:(tn + 1) * TN],
                                     start=True, stop=True)
                    dsl = delta[:, g, tn * TN:(tn + 1) * TN]
                    nc.scalar.activation(out=dsl, in_=psd[:],
                                         func=AF.Exp, bias=bd[:, g:g + 1], scale=1.0)
                if g == 3:
                    nc.scalar.activation(out=delta[:, 0:4, :], in_=delta[:, 0:4, :],
                                         func=AF.Ln, bias=1.0, scale=1.0)
                    nc.gpsimd.tensor_tensor(out=du[:, 0:4, :], in0=delta[:, 0:4, :],
                                            in1=u3[:, 0:4, :], op=OP.mult)
            nc.scalar.activation(out=delta[:, 4:8, :], in_=delta[:, 4:8, :],
                                 func=AF.Ln, bias=1.0, scale=1.0)
            nc.gpsimd.tensor_tensor(out=du[:, 4:8, :], in0=delta[:, 4:8, :],
                                    in1=u3[:, 4:8, :], op=OP.mult)
        poolU_cm.__exit__(None, None, None)

        # ---------------- Phase B: selective scan (block-outer) ----------------
        with tc.tile_pool(name="scan", bufs=2) as sp, \
             tc.tile_pool(name="quadp", bufs=1) as quadp, \
             tc.tile_pool(name="scanA", bufs=2) as spA, \
             tc.tile_pool(name="scan3", bufs=2) as sp3, \
             tc.tile_pool(name="oproj", bufs=1) as op_pool, \
             tc.tile_pool(name="osbp", bufs=3) as osbp, \
             tc.tile_pool(name="oc", bufs=1) as oc_pool, \
             tc.tile_pool(name="op_ps", bufs=4, space="PSUM") as op_ps:
            wco_r = wco.ap().rearrange("(k p) m -> p k m", p=128)
            outp_r = outp.ap().rearrange("b (m p) t -> b p m t", p=128)
            for blk in range(NB):
                g0 = blk * g_blk
                quad = None
                for n in range(NST):
                    brep = sp3.tile([128, L], scan_dt, tag="brep")
                    nc.sync.dma_start(out=brep[:], in_=cc_bf[n: n + 1, :].to_broadcast((128, L)))
                    crep = sp3.tile([128, L], scan_dt, tag="crep")
                    nc.sync.dma_start(out=crep[:], in_=cc_bf[16 + n: 17 + n, :].to_broadcast((128, L)))
                    dA = spA.tile([128, FB], scan_dt, tag="dA")
                    if structured_a:
                        nc.scalar.activation(
                            out=dA[:].rearrange("p (g t) -> p g t", g=g_blk),
                            in_=delta[:, g0:g0 + g_blk, :],
                            func=AF.Exp, bias=0.0, scale=At[:, 0, n:n + 1])
                    else:
                        for gg in range(g_blk):
                            nc.scalar.activation(
                                out=dA[:, gg * L:(gg + 1) * L],
                                in_=delta[:, g0 + gg, :],
                                func=AF.Exp, bias=0.0, scale=At[:, g0 + gg, n:n + 1])
                    dAz = bass.AP(tensor=dA.tensor, offset=dA.offset,
                                  ap=[[dA.ap[0][0], 128], [L, g_blk], [1, 1]])
                    nc.gpsimd.memset(dAz, 0.0)
                    dBu = sp.tile([128, FB], scan_dt, tag="dBu")
                    bap = bass.AP(tensor=brep.tensor, offset=brep.offset,
                                  ap=[[brep.ap[0][0], 128], [0, g_blk], [1, L]])
                    dbu_eng = nc.gpsimd if (n % 4 != 3) else nc.vector
                    dbu_eng.tensor_tensor(
                        out=dBu[:].rearrange("p (g t) -> p g t", g=g_blk),
                        in0=du[:, g0:g0 + g_blk, :], in1=bap, op=OP.mult)
                    h = sp.tile([128, FB], scan_dt, tag="h")
                    nc.vector.tensor_tensor_scan(h[:], dA[:], dBu[:], 0.0, OP.mult, OP.add)
                    # hc computed in place over h
                    cap = bass.AP(tensor=crep.tensor, offset=crep.offset,
                                  ap=[[crep.ap[0][0], 128], [0, g_blk], [1, L]])
                    ph = n % 4
                    hc_eng = nc.gpsimd if ph in (0, 1, 2) else nc.vector
                    h3 = h[:].rearrange("p (g t) -> p g t", g=g_blk)
                    hc_eng.tensor_tensor(out=h3, in0=h3, in1=cap, op=OP.mult)
                    if ph == 0:
                        quad = quadp.tile([128, FB], scan_dt, tag="quad")
                        nc.gpsimd.tensor_copy(quad[:], h[:])
                    elif ph == 1:
                        nc.gpsimd.tensor_tensor(out=quad[:], in0=quad[:], in1=h[:], op=OP.add)
                    else:
                        nc.vector.tensor_tensor(out=quad[:], in0=quad[:], in1=h[:], op=OP.add)
                    if ph == 3:
                        nc.vector.tensor_tensor(
                            out=yac[:, g0:g0 + g_blk, :],
                            in0=yac[:, g0:g0 + g_blk, :],
                            in1=quad[:].rearrange("p (g t) -> p g t", g=g_blk), op=OP.add)
                # gate this block and emit its partial output projection
                ygb = op_pool.tile([128, g_blk, L], BF16, tag="ygb")
                nc.vector.tensor_tensor(out=ygb[:], in0=yac[:, g0:g0 + g_blk, :],
                                        in1=szg[:, g0:g0 + g_blk, :], op=OP.mult)
                for mc in range(2):
                    wc = oc_pool.tile([128, 8, 512], BF16, tag="wc")
                    nc.gpsimd.dma_start(out=wc[:], in_=wco_r[:, :, mc * 512:(mc + 1) * 512])
                    for mm in range(4):
                        m = mc * 4 + mm
                        osl = osbp.tile([128, L], F32, tag="osl")
                        for tn in range(NT):
                            ps = op_ps.tile([128, TN], F32, tag="ps_o")
                            for kk in range(g_blk):
                                nc.tensor.matmul(ps[:], wc[:, g0 + kk, mm * 128:(mm + 1) * 128],
                                                 ygb[:, kk, tn * TN:(tn + 1) * TN],
                                                 start=(kk == 0), stop=(kk == g_blk - 1))
                            nc.scalar.copy(out=osl[:, tn * TN:(tn + 1) * TN], in_=ps[:])
                        nc.sync.dma_start(out=outp_r[blk, :, m, :], in_=osl[:])
        poolD_cm.__exit__(None, None, None)
        szgp_cm.__exit__(None, None, None)
        yacp_cm.__exit__(None, None, None)
        consts_cm.__exit__(None, None, None)

    nc.compile()
    return nc


def _prep_core_inputs(inputs, b, d, h):
    pref = "f_" if d == 0 else "b_"
    g = lambda k: np.asarray(inputs[pref + k], dtype=np.float32)
    x = np.asarray(inputs["x"], dtype=np.float32)[b]
    if d == 1:
        x = x[::-1]
    sl = slice(h * DH, (h + 1) * DH)

    Win = g("Win")
    w_in = np.concatenate([Win[sl].T, Win[2048 + h * DH: 2048 + (h + 1) * DH].T], axis=1)
    convw = g("convw")[sl]
    convb_eff = g("convb")[sl] + g("bin")[sl] * convw.sum(-1)
    pg = lambda v: np.ascontiguousarray(v.reshape(NG, 128).T)
    pg3 = lambda v: np.ascontiguousarray(v.reshape(NG, 128, -1).transpose(1, 0, 2))
    A = -np.exp(g("Alog")[sl])
    proj_W = np.asarray(inputs["proj_W"], dtype=np.float32)
    Pd = proj_W[:, d * DM:(d + 1) * DM]
    wco = (Pd @ g("Wout"))[:, sl].T
    return {
        "xT": np.ascontiguousarray(x.T),
        "w_in": np.ascontiguousarray(w_in),
        "wx": np.ascontiguousarray(g("Wx")[:, sl].T),
        "wdt": np.ascontiguousarray(g("Wdt")[sl].T),
        "wco": np.ascontiguousarray(wco),
        "convw": pg3(convw),
        "convb": pg(convb_eff),
        "binz": pg(g("bin")[2048 + h * DH: 2048 + (h + 1) * DH]),
        "bdt": pg(g("bdt")[sl]),
        "A_": pg3(A),
        "dsk": pg(g("Dsk")[sl]),
    }


def _check_structured_a(inputs):
    ar = np.log(np.arange(1, NST + 1, dtype=np.float32))
    for pref in ("f_", "b_"):
        Alog = np.asarray(inputs[pref + "Alog"], dtype=np.float32)
        if not np.allclose(Alog, np.broadcast_to(ar, Alog.shape), atol=1e-5):
            return False
    return True


_CACHE = {}


def _get_nc(structured_a):
    key = ("v1", structured_a)
    if key not in _CACHE:
        _CACHE[key] = _build(structured_a)
    return _CACHE[key]


def kernel(**inputs):
    from concourse.bass_utils import run_bass_kernel_spmd

    nc = _get_nc(_check_structured_a(inputs))
    in_maps = []
    for c in range(N_CORES):
        b, d, h = c >> 2, (c >> 1) & 1, c & 1
        in_maps.append(_prep_core_inputs(inputs, b, d, h))
    res = run_bass_kernel_spmd(nc, in_maps, list(range(N_CORES)))
    partials = [res.results[c]["outp"].sum(axis=0) for c in range(N_CORES)]

    B = 2
    out = np.zeros((B, L, DM), np.float32)
    for b in range(B):
        for d in range(2):
            s = (partials[b * 4 + d * 2 + 0] + partials[b * 4 + d * 2 + 1]).T
            if d == 1:
                s = s[::-1]
            out[b] += s
    proj_W = np.asarray(inputs["proj_W"], dtype=np.float32)
    bias = (np.asarray(inputs["f_bout"], dtype=np.float32) @ proj_W[:, :DM].T
            + np.asarray(inputs["b_bout"], dtype=np.float32) @ proj_W[:, DM:].T
            + np.asarray(inputs["proj_b"], dtype=np.float32))
    return out + bias



# revision 18
# speedup vs baseline: 51.7420x; 51.7420x over previous
"""BiSSM (bidirectional Mamba block) Trainium2 kernel, v2.

Contract: kernel(**inputs) takes the FULL unsharded inputs of
nn_BiSSMBlock (see setup_inputs) and returns the full (2, 1024, 1024)
output.  Sharding: (batch 2) x (direction 2) x (d_inner half 2) over 8
cores; a pairwise bf16 AllReduce between half-cores for the x_dbl
projection.  Host folds Wout into proj_W per direction and sums/flips
partials.

v2 layout vs v1: depthwise conv and the per-state y accumulation run on
the PE array (diagonal / identity matmuls accumulating in PSUM), scans
are split between DVE and GpSimd, the collective payload is bf16 and
overlaps the z-half of in_proj, and the output projection emits bf16
block partials.

Self-contained: only needs the concourse/bass toolchain at
/opt/trn_rl_repo and 8 visible neuron cores.
"""
import sys
sys.path.insert(0, "/opt/trn_rl_repo")
import numpy as np

import concourse.bass as bass
import concourse.bacc as bacc
import concourse.mybir as mybir
import concourse.tile as tile

F32 = mybir.dt.float32
F32R = mybir.dt.float32r
BF16 = mybir.dt.bfloat16
OP = mybir.AluOpType
AF = mybir.ActivationFunctionType

L = 1024          # sequence length
DM = 1024         # d_model
DH = 1024         # d_inner half per core
NG = 8            # channel groups of 128 (DH/128)
NST = 16          # d_state
TN = 512          # matmul moving-dim tile
NB = 4            # phase-B blocks
GB = 2            # groups per block

N_CORES = 8
CC_GROUPS = [[0, 1], [2, 3], [4, 5], [6, 7]]


def _build(structured_a, repeat=1, n_cores=N_CORES):
    nc = bacc.Bacc("TRN2", target_bir_lowering=False, debug=False, num_devices=n_cores)

    xT = nc.declare_dram_parameter("xT", [DM, L], BF16, isOutput=False)
    w_in = nc.declare_dram_parameter("w_in", [DM, 2 * DH], BF16, isOutput=False)
    wx = nc.declare_dram_parameter("wx", [DH, 96], BF16, isOutput=False)
    wdt = nc.declare_dram_parameter("wdt", [64, DH], BF16, isOutput=False)
    wco = nc.declare_dram_parameter("wco", [DH, DM], BF16, isOutput=False)
    convw = nc.declare_dram_parameter("convw", [128, NG, 4], F32, isOutput=False)
    dskdiag = nc.declare_dram_parameter("dskdiag", [128, NG, 128], BF16, isOutput=False)
    ident = nc.declare_dram_parameter("ident", [128, 128], BF16, isOutput=False)
    convb = nc.declare_dram_parameter("convb", [128, NG], F32, isOutput=False)
    binz = nc.declare_dram_parameter("binz", [128, NG], F32, isOutput=False)
    bdt = nc.declare_dram_parameter("bdt", [128, NG], F32, isOutput=False)
    A_ = nc.declare_dram_parameter("A_", [128, NG, NST], F32, isOutput=False)
    outp = nc.declare_dram_parameter("outp", [NB, DM, L], BF16, isOutput=True)

    ccs = [(nc.dram_tensor(f"cc_in{r}", [96, L], BF16),
            nc.dram_tensor(f"cc_ag{r}", [2 * 96, L], BF16),
            nc.dram_tensor(f"cc_sum{r}", [32, L], BF16)) for r in range(repeat)]

    with tile.TileContext(nc) as tc:
        consts_cm = tc.tile_pool(name="consts", bufs=1)
        consts = consts_cm.__enter__()
        cw = consts.tile([128, NG, 4], F32)
        nc.sync.dma_start(out=cw[:], in_=convw[:])
        ddg = consts.tile([128, NG, 128], BF16)
        nc.sync.dma_start(out=ddg[:], in_=dskdiag[:])
        idt = consts.tile([128, 128], BF16)
        nc.sync.dma_start(out=idt[:], in_=ident[:])
        cb = consts.tile([128, NG], F32)
        nc.sync.dma_start(out=cb[:], in_=convb[:])
        bz = consts.tile([128, NG], F32)
        nc.sync.dma_start(out=bz[:], in_=binz[:])
        bd = consts.tile([128, NG], F32)
        nc.sync.dma_start(out=bd[:], in_=bdt[:])
        At = consts.tile([128, NG, NST], F32)
        nc.sync.dma_start(out=At[:], in_=A_[:])

        for rep in range(repeat):
            cc_in, cc_ag, cc_sum = ccs[rep]
            _emit_one(nc, tc, structured_a, rep,
                      xT, w_in, wx, wdt, wco, outp, cc_in, cc_ag, cc_sum,
                      cw, ddg, idt, cb, bz, bd, At)

        consts_cm.__exit__(None, None, None)

    nc.compile()
    return nc


def _emit_one(nc, tc, structured_a, rep,
              xT, w_in, wx, wdt, wco, outp, cc_in, cc_ag, cc_sum,
              cw, ddg, idt, cb, bz, bd, At):
    w_in_r = w_in.ap().rearrange("(k p) m -> p k m", p=128)
    sfx = f"_{rep}"

    bigp_cm = tc.tile_pool(name="bigp" + sfx, bufs=1)
    bigp = bigp_cm.__enter__()
    u3 = bigp.tile([128, NG, L], BF16, tag="u3")
    szg = bigp.tile([128, NG, L], BF16, tag="szg")
    delta = bigp.tile([128, NG, L], F32, tag="delta")
    du = bigp.tile([128, NG, L], BF16, tag="du")

    # ---------------- Phase A: projections + conv ----------------
    poolA_cm = tc.tile_pool(name="poolA" + sfx, bufs=1)
    poolA = poolA_cm.__enter__()
    with tc.tile_pool(name="wchunk" + sfx, bufs=2) as wchunk, \
         tc.tile_pool(name="scrp" + sfx, bufs=2) as scrp, \
         tc.tile_pool(name="mm_ps" + sfx, bufs=4, space="PSUM") as mm_ps:
        xts = poolA.tile([128, 8, L], BF16, tag="xts")
        for k in range(8):
            nc.sync.dma_start(
                out=xts[:, k, :],
                in_=xT.ap().rearrange("(k p) t -> p k t", p=128)[:, k, :])
        xin = poolA.tile([128, NG, L + 3], BF16, tag="xin")
        halo = bass.AP(tensor=xin.tensor, offset=xin.offset,
                       ap=[[xin.ap[0][0], 128], [L + 3, NG], [1, 3]])
        nc.vector.memset(halo, 0.0)

        # xin half of in_proj, then depthwise conv via diagonal matmuls
        for m in range(8):
            wi = wchunk.tile([128, 8, 128], BF16, tag="wi")
            nc.sync.dma_start(out=wi[:], in_=w_in_r[:, :, m * 128:(m + 1) * 128])
            for tn in range(2):
                ps = mm_ps.tile([128, TN], F32, tag="ps")
                for k in range(8):
                    nc.tensor.matmul(ps[:], wi[:, k, :],
                                     xts[:, k, tn * TN:(tn + 1) * TN],
                                     start=(k == 0), stop=(k == 7))
                nc.scalar.copy(out=xin[:, m, 3 + tn * TN: 3 + (tn + 1) * TN], in_=ps[:])
            cv_eng = nc.vector
            scr = scrp.tile([128, L], F32, tag="scr")
            cv_eng.tensor_scalar_mul(out=scr[:], in0=xin[:, m, 3:3 + L],
                                     scalar1=cw[:, m, 3:4])
            for k in range(3):
                cv_eng.scalar_tensor_tensor(
                    out=scr[:], in0=xin[:, m, k:k + L],
                    scalar=cw[:, m, k:k + 1], in1=scr[:],
                    op0=OP.mult, op1=OP.add)
            nc.scalar.activation(out=u3[:, m, :], in_=scr[:],
                                 func=AF.Silu, bias=cb[:, m:m + 1], scale=1.0)

        # x_dbl partial projection -> cc_in (bf16)
        wxs = poolA.tile([128, 8, 96], BF16, tag="wxs")
        nc.sync.dma_start(out=wxs[:], in_=wx.ap().rearrange("(k p) m -> p k m", p=128))
        xdb = poolA.tile([96, L], BF16, tag="xdb")
        for tn in range(2):
            ps96 = mm_ps.tile([96, TN], F32, tag="ps96", bufs=2)
            for k in range(8):
                nc.tensor.matmul(ps96[:], wxs[:, k, :],
                                 u3[:, k, tn * TN:(tn + 1) * TN],
                                 start=(k == 0), stop=(k == 7))
            nc.scalar.copy(out=xdb[:, tn * TN:(tn + 1) * TN], in_=ps96[:])
        nc.sync.dma_start(out=cc_in[:], in_=xdb[:])

        # z half of in_proj: emitted before the collective so its PE/ACT
        # work fills the AllReduce wait window
        for m in range(8):
            wi = wchunk.tile([128, 8, 128], BF16, tag="wi")
            nc.sync.dma_start(out=wi[:], in_=w_in_r[:, :, DH + m * 128: DH + (m + 1) * 128])
            for tn in range(2):
                ps = mm_ps.tile([128, TN], F32, tag="ps")
                for k in range(8):
                    nc.tensor.matmul(ps[:], wi[:, k, :],
                                     xts[:, k, tn * TN:(tn + 1) * TN],
                                     start=(k == 0), stop=(k == 7))
                nc.scalar.activation(out=szg[:, m, tn * TN:(tn + 1) * TN],
                                     in_=ps[:], func=AF.Silu,
                                     bias=bz[:, m:m + 1], scale=1.0)

        nc.gpsimd.collective_compute("AllGather", OP.bypass, replica_groups=CC_GROUPS,
                                     ins=[cc_in[:]], outs=[cc_ag[:]])
        agt = poolA.tile([96, 2, L], BF16, tag="agt")
        nc.sync.dma_start(out=agt[:, 0, :],
                          in_=bass.AP(tensor=cc_ag, offset=0, ap=[[L, 96], [1, L]]))
        nc.scalar.dma_start(out=agt[:, 1, :],
                            in_=bass.AP(tensor=cc_ag, offset=96 * L, ap=[[L, 96], [1, L]]))
        xsum = poolA.tile([32, L], BF16, tag="xsum")
        nc.vector.tensor_tensor(out=xsum[:], in0=agt[64:96, 0, :], in1=agt[64:96, 1, :],
                                op=OP.add)
        nc.sync.dma_start(out=cc_sum[:], in_=xsum[:])

        # delta = softplus(dt @ Wdt.T + bdt), via exp then ln(1+x)
        wds = poolA.tile([64, NG, 128], BF16, tag="wds")
        nc.sync.dma_start(out=wds[:], in_=wdt.ap().rearrange("k (g p) -> k g p", p=128))
        for g in range(NG):
            for tn in range(2):
                psd = mm_ps.tile([128, TN], F32, tag="ps")
                for s in range(2):
                    nc.tensor.matmul(psd[:], wds[:, g, :],
                                     agt[0:64, s, tn * TN:(tn + 1) * TN],
                                     start=(s == 0), stop=(s == 1))
                nc.scalar.activation(out=delta[:, g, tn * TN:(tn + 1) * TN],
                                     in_=psd[:], func=AF.Exp, bias=bd[:, g:g + 1], scale=1.0)
            if g % 2 == 1:
                nc.scalar.activation(out=delta[:, g - 1:g + 1, :], in_=delta[:, g - 1:g + 1, :],
                                     func=AF.Ln, bias=1.0, scale=1.0)
                nc.vector.tensor_tensor(out=du[:, g - 1:g + 1, :], in0=delta[:, g - 1:g + 1, :],
                                        in1=u3[:, g - 1:g + 1, :], op=OP.mult)
    poolA_cm.__exit__(None, None, None)

    # ---------------- Phase B: selective scan + out_proj ----------------
    with tc.tile_pool(name="wcp" + sfx, bufs=1) as wcp, \
         tc.tile_pool(name="bcpool" + sfx, bufs=6) as bcpool, \
         tc.tile_pool(name="sA" + sfx, bufs=2) as sA, \
         tc.tile_pool(name="dAkp" + sfx, bufs=2) as dAkp, \
         tc.tile_pool(name="sB" + sfx, bufs=3) as sB, \
         tc.tile_pool(name="sH" + sfx, bufs=4) as sH, \
         tc.tile_pool(name="ygbp" + sfx, bufs=2) as ygbp, \
         tc.tile_pool(name="oslp" + sfx, bufs=4) as oslp, \
         tc.tile_pool(name="ps_y" + sfx, bufs=4, space="PSUM") as psy_pool, \
         tc.tile_pool(name="op_ps" + sfx, bufs=4, space="PSUM") as op_ps:
        wco_r = wco.ap().rearrange("(k p) m -> p k m", p=128)
        outp_r = outp.ap().rearrange("b (m p) t -> b p m t", p=128)
        wc = []
        for mc in range(2):
            w_ = wcp.tile([128, 8, TN], BF16, tag=f"wc{mc}")
            nc.scalar.dma_start(out=w_[:], in_=wco_r[:, :, mc * TN:(mc + 1) * TN])
            wc.append(w_)

        idx = 0
        for blk in range(NB):
            g0 = blk * GB
            dAkeep = [None] * 4
            psy = [psy_pool.tile([128, TN], F32, tag="psy", name=f"psy{rep}_{blk}_{i}")
                   for i in range(4)]
            # tslice ts covers (group g0 + ts//2, columns (ts%2)*TN)
            for ts in range(4):
                g = g0 + ts // 2
                t0 = (ts % 2) * TN
                nc.tensor.matmul(psy[ts][:], ddg[:, g, :], u3[:, g, t0:t0 + TN],
                                 start=True, stop=False)
            for n in range(NST):
                bcp = bcpool.tile([128, 2, L], BF16, tag="bcp")
                nc.sync.dma_start(
                    out=bcp[:],
                    in_=bass.AP(tensor=cc_sum, offset=2 * n * L,
                                ap=[[0, 128], [L, 2], [1, L]]))
                brep = bass.AP(tensor=bcp.tensor, offset=bcp.offset,
                               ap=[[bcp.ap[0][0], 128], [0, GB], [1, L]])
                crep = bass.AP(tensor=bcp.tensor, offset=bcp.offset + L,
                               ap=[[bcp.ap[0][0], 128], [0, GB], [1, L]])
                if structured_a and n in (4, 5, 6, 7):
                    dA = dAkp.tile([128, GB, L], BF16, tag=f"dAk{n - 4}",
                                   name=f"dAk{rep}_{blk}_{n}")
                    dAkeep[n - 4] = dA
                else:
                    dA = sA.tile([128, GB, L], BF16, tag="dA")
                if structured_a and n in (9, 11, 13, 15):
                    src_dA = dAkeep[(n - 9) // 2]
                    sq_eng = nc.vector if n in (9, 13) else nc.gpsimd
                    sq_eng.tensor_tensor(out=dA[:], in0=src_dA[:], in1=src_dA[:],
                                         op=OP.mult)
                elif structured_a:
                    nc.scalar.activation(out=dA[:], in_=delta[:, g0:g0 + GB, :],
                                         func=AF.Exp, bias=0.0, scale=-float(n + 1))
                else:
                    for gg in range(GB):
                        nc.scalar.activation(out=dA[:, gg, :], in_=delta[:, g0 + gg, :],
                                             func=AF.Exp, bias=0.0,
                                             scale=At[:, g0 + gg, n:n + 1])
                dBu = sB.tile([128, GB, L], BF16, tag="dBu")
                nc.gpsimd.tensor_tensor(out=dBu[:], in0=du[:, g0:g0 + GB, :],
                                        in1=brep, op=OP.mult)
                h = sH.tile([128, GB, L], BF16, tag="h")
                for gg in range(GB):
                    nc.vector.tensor_tensor_scan(h[:, gg, :], dA[:, gg, :], dBu[:, gg, :],
                                                 0.0, OP.mult, OP.add)
                hc_eng = nc.gpsimd if (idx % 5 != 1) else nc.vector
                hc_eng.tensor_tensor(out=h[:], in0=h[:], in1=crep, op=OP.mult)
                for ts in range(4):
                    nc.tensor.matmul(psy[ts][:], idt[:],
                                     h[:, ts // 2, (ts % 2) * TN:(ts % 2) * TN + TN],
                                     start=False, stop=(n == NST - 1))
                idx += 1
            # gate with silu(z) and project this block's channels
            ygb = ygbp.tile([128, GB, L], BF16, tag="ygb")
            for ts in range(4):
                g = g0 + ts // 2
                t0 = (ts % 2) * TN
                nc.vector.tensor_tensor(out=ygb[:, ts // 2, t0:t0 + TN], in0=psy[ts][:],
                                          in1=szg[:, g, t0:t0 + TN], op=OP.mult)
            for mc in range(2):
                for mm in range(4):
                    m = mc * 4 + mm
                    for tn in range(2):
                        ps = op_ps.tile([128, TN], F32, tag="ps_o")
                        for kk in range(GB):
                            nc.tensor.matmul(ps[:], wc[mc][:, g0 + kk, mm * 128:(mm + 1) * 128],
                                             ygb[:, kk, tn * TN:(tn + 1) * TN],
                                             start=(kk == 0), stop=(kk == GB - 1))
                        osl = oslp.tile([128, TN], BF16, tag="osl")
                        if (mm + tn) % 2 == 0:
                            nc.scalar.copy(out=osl[:], in_=ps[:])
                        else:
                            nc.vector.tensor_copy(osl[:], ps[:])
                        nc.sync.dma_start(out=outp_r[blk, :, m, tn * TN:(tn + 1) * TN],
                                          in_=osl[:])
    bigp_cm.__exit__(None, None, None)


def _prep_core_inputs(inputs, b, d, h):
    pref = "f_" if d == 0 else "b_"
    g = lambda k: np.asarray(inputs[pref + k], dtype=np.float32)
    x = np.asarray(inputs["x"], dtype=np.float32)[b]
    if d == 1:
        x = x[::-1]
    sl = slice(h * DH, (h + 1) * DH)

    Win = g("Win")
    w_in = np.concatenate([Win[sl].T, Win[2048 + h * DH: 2048 + (h + 1) * DH].T], axis=1)
    convw = g("convw")[sl]
    convb_eff = g("convb")[sl] + g("bin")[sl] * convw.sum(-1)
    pg = lambda v: np.ascontiguousarray(v.reshape(NG, 128).T)
    pg3 = lambda v: np.ascontiguousarray(v.reshape(NG, 128, -1).transpose(1, 0, 2))
    A = -np.exp(g("Alog")[sl])
    proj_W = np.asarray(inputs["proj_W"], dtype=np.float32)
    Pd = proj_W[:, d * DM:(d + 1) * DM]
    wco = (Pd @ g("Wout"))[:, sl].T

    # x_dbl rows permuted: dt 0:64 unchanged, then B/C interleaved
    Wx = g("Wx")[:, sl]
    perm = list(range(64)) + [64 + 16 * (i % 2) + i // 2 for i in range(32)]
    Wx = Wx[perm]

    # diagonal weight matrices for the D-skip matmuls
    dskdiag = np.zeros((128, NG, 128), np.float32)
    dk = pg(g("Dsk")[sl])    # [128, NG]
    rng = np.arange(128)
    for gi in range(NG):
        dskdiag[rng, gi, rng] = dk[:, gi]

    import ml_dtypes
    bf = lambda v: np.ascontiguousarray(v).astype(ml_dtypes.bfloat16)
    return {
        "xT": bf(x.T),
        "w_in": bf(w_in),
        "wx": bf(Wx.T),
        "wdt": bf(g("Wdt")[sl].T),
        "wco": bf(wco),
        "convw": pg3(convw),
        "dskdiag": bf(dskdiag),
        "ident": bf(np.eye(128, dtype=np.float32)),
        "convb": pg(convb_eff),
        "binz": pg(g("bin")[2048 + h * DH: 2048 + (h + 1) * DH]),
        "bdt": pg(g("bdt")[sl]),
        "A_": pg3(A),
    }


def _check_structured_a(inputs):
    ar = np.log(np.arange(1, NST + 1, dtype=np.float32))
    for pref in ("f_", "b_"):
        Alog = np.asarray(inputs[pref + "Alog"], dtype=np.float32)
        if not np.allclose(Alog, np.broadcast_to(ar, Alog.shape), atol=1e-5):
            return False
    return True


_CACHE = {}


def _get_nc(structured_a, repeat=1):
    key = ("v2", structured_a, repeat)
    if key not in _CACHE:
        _CACHE[key] = _build(structured_a, repeat=repeat)
    return _CACHE[key]


def kernel(**inputs):
    from concourse.bass_utils import run_bass_kernel_spmd

    nc = _get_nc(_check_structured_a(inputs))
    in_maps = []
    for c in range(N_CORES):
        b, d, h = c >> 2, (c >> 1) & 1, c & 1
        in_maps.append(_prep_core_inputs(inputs, b, d, h))
    res = run_bass_kernel_spmd(nc, in_maps, list(range(N_CORES)))
    partials = [np.asarray(res.results[c]["outp"], dtype=np.float32).sum(axis=0)
                for c in range(N_CORES)]

    B = 2
    out = np.zeros((B, L, DM), np.float32)
    for b in range(B):
        for d in range(2):
            s = (partials[b * 4 + d * 2 + 0] + partials[b * 4 + d * 2 + 1]).T
            if d == 1:
                s = s[::-1]
            out[b] += s
    proj_W = np.asarray(inputs["proj_W"], dtype=np.float32)
    bias = (np.asarray(inputs["f_bout"], dtype=np.float32) @ proj_W[:, :DM].T
            + np.asarray(inputs["b_bout"], dtype=np.float32) @ proj_W[:, DM:].T
            + np.asarray(inputs["proj_b"], dtype=np.float32))
    return out + bias


# revision 19
# speedup vs baseline: 53.1520x; 1.0273x over previous
"""BiSSM (bidirectional Mamba block) Trainium2 kernel, v2.

Contract: kernel(**inputs) takes the FULL unsharded inputs of
nn_BiSSMBlock (see setup_inputs) and returns the full (2, 1024, 1024)
output.  Sharding: (batch 2) x (direction 2) x (d_inner half 2) over 8
cores; a pairwise bf16 AllReduce between half-cores for the x_dbl
projection.  Host folds Wout into proj_W per direction and sums/flips
partials.

v2 layout vs v1: depthwise conv and the per-state y accumulation run on
the PE array (diagonal / identity matmuls accumulating in PSUM), scans
are split between DVE and GpSimd, the collective payload is bf16 and
overlaps the z-half of in_proj, and the output projection emits bf16
block partials.

Self-contained: only needs the concourse/bass toolchain at
/opt/trn_rl_repo and 8 visible neuron cores.
"""
import sys
sys.path.insert(0, "/opt/trn_rl_repo")
import numpy as np

import concourse.bass as bass
import concourse.bacc as bacc
import concourse.mybir as mybir
import concourse.tile as tile

F32 = mybir.dt.float32
F32R = mybir.dt.float32r
BF16 = mybir.dt.bfloat16
OP = mybir.AluOpType
AF = mybir.ActivationFunctionType

L = 1024          # sequence length
DM = 1024         # d_model
DH = 1024         # d_inner half per core
NG = 8            # channel groups of 128 (DH/128)
NST = 16          # d_state
TN = 512          # matmul moving-dim tile
NB = 4            # phase-B blocks
GB = 2            # groups per block

N_CORES = 8
CC_GROUPS = [[0, 1], [2, 3], [4, 5], [6, 7]]


def _build(structured_a, repeat=1, n_cores=N_CORES):
    nc = bacc.Bacc("TRN2", target_bir_lowering=False, debug=False, num_devices=n_cores)

    xT = nc.declare_dram_parameter("xT", [DM, L], BF16, isOutput=False)
    w_in = nc.declare_dram_parameter("w_in", [DM, 2 * DH], BF16, isOutput=False)
    wx = nc.declare_dram_parameter("wx", [DH, 96], BF16, isOutput=False)
    wdt = nc.declare_dram_parameter("wdt", [64, DH], BF16, isOutput=False)
    wco = nc.declare_dram_parameter("wco", [DH, DM], BF16, isOutput=False)
    convw = nc.declare_dram_parameter("convw", [128, NG, 4], F32, isOutput=False)
    dskdiag = nc.declare_dram_parameter("dskdiag", [128, NG, 128], BF16, isOutput=False)
    ident = nc.declare_dram_parameter("ident", [128, 128], BF16, isOutput=False)
    convb = nc.declare_dram_parameter("convb", [128, NG], F32, isOutput=False)
    binz = nc.declare_dram_parameter("binz", [128, NG], F32, isOutput=False)
    bdt = nc.declare_dram_parameter("bdt", [128, NG], F32, isOutput=False)
    A_ = nc.declare_dram_parameter("A_", [128, NG, NST], F32, isOutput=False)
    outp = nc.declare_dram_parameter("outp", [NB, DM, L], BF16, isOutput=True)

    ccs = [(nc.dram_tensor(f"cc_in{r}", [96, L], F32),
            nc.dram_tensor(f"cc_out{r}", [96, L], F32),
            nc.dram_tensor(f"cc_sum{r}", [32, L], BF16)) for r in range(repeat)]

    with tile.TileContext(nc) as tc:
        consts_cm = tc.tile_pool(name="consts", bufs=1)
        consts = consts_cm.__enter__()
        cw = consts.tile([128, NG, 4], F32)
        nc.sync.dma_start(out=cw[:], in_=convw[:])
        ddg = consts.tile([128, NG, 128], BF16)
        nc.sync.dma_start(out=ddg[:], in_=dskdiag[:])
        idt = consts.tile([128, 128], BF16)
        nc.sync.dma_start(out=idt[:], in_=ident[:])
        cb = consts.tile([128, NG], F32)
        nc.sync.dma_start(out=cb[:], in_=convb[:])
        bz = consts.tile([128, NG], F32)
        nc.sync.dma_start(out=bz[:], in_=binz[:])
        bd = consts.tile([128, NG], F32)
        nc.sync.dma_start(out=bd[:], in_=bdt[:])
        At = consts.tile([128, NG, NST], F32)
        nc.sync.dma_start(out=At[:], in_=A_[:])

        for rep in range(repeat):
            cc_in, cc_out, cc_sum = ccs[rep]
            _emit_one(nc, tc, structured_a, rep,
                      xT, w_in, wx, wdt, wco, outp, cc_in, cc_out, cc_sum,
                      cw, ddg, idt, cb, bz, bd, At)

        consts_cm.__exit__(None, None, None)

    nc.compile()
    return nc


def _emit_one(nc, tc, structured_a, rep,
              xT, w_in, wx, wdt, wco, outp, cc_in, cc_out, cc_sum,
              cw, ddg, idt, cb, bz, bd, At):
    w_in_r = w_in.ap().rearrange("(k p) m -> p k m", p=128)
    sfx = f"_{rep}"

    bigp_cm = tc.tile_pool(name="bigp" + sfx, bufs=1)
    bigp = bigp_cm.__enter__()
    u3 = bigp.tile([128, NG, L], BF16, tag="u3")
    szg = bigp.tile([128, NG, L], BF16, tag="szg")
    delta = bigp.tile([128, NG, L], F32, tag="delta")
    du = bigp.tile([128, NG, L], BF16, tag="du")

    # ---------------- Phase A: projections + conv ----------------
    poolA_cm = tc.tile_pool(name="poolA" + sfx, bufs=1)
    poolA = poolA_cm.__enter__()
    with tc.tile_pool(name="wchunk" + sfx, bufs=2) as wchunk, \
         tc.tile_pool(name="scrp" + sfx, bufs=2) as scrp, \
         tc.tile_pool(name="mm_ps" + sfx, bufs=4, space="PSUM") as mm_ps:
        xts = poolA.tile([128, 8, L], BF16, tag="xts")
        for k in range(8):
            nc.sync.dma_start(
                out=xts[:, k, :],
                in_=xT.ap().rearrange("(k p) t -> p k t", p=128)[:, k, :])
        xin = poolA.tile([128, NG, L + 3], BF16, tag="xin")
        halo = bass.AP(tensor=xin.tensor, offset=xin.offset,
                       ap=[[xin.ap[0][0], 128], [L + 3, NG], [1, 3]])
        nc.vector.memset(halo, 0.0)

        # xin half of in_proj, then depthwise conv via diagonal matmuls
        for m in range(8):
            wi = wchunk.tile([128, 8, 128], BF16, tag="wi")
            nc.sync.dma_start(out=wi[:], in_=w_in_r[:, :, m * 128:(m + 1) * 128])
            for tn in range(2):
                ps = mm_ps.tile([128, TN], F32, tag="ps")
                for k in range(8):
                    nc.tensor.matmul(ps[:], wi[:, k, :],
                                     xts[:, k, tn * TN:(tn + 1) * TN],
                                     start=(k == 0), stop=(k == 7))
                nc.scalar.copy(out=xin[:, m, 3 + tn * TN: 3 + (tn + 1) * TN], in_=ps[:])
            cv_eng = nc.vector
            scr = scrp.tile([128, L], F32, tag="scr")
            cv_eng.tensor_scalar_mul(out=scr[:], in0=xin[:, m, 3:3 + L],
                                     scalar1=cw[:, m, 3:4])
            for k in range(3):
                cv_eng.scalar_tensor_tensor(
                    out=scr[:], in0=xin[:, m, k:k + L],
                    scalar=cw[:, m, k:k + 1], in1=scr[:],
                    op0=OP.mult, op1=OP.add)
            nc.scalar.activation(out=u3[:, m, :], in_=scr[:],
                                 func=AF.Silu, bias=cb[:, m:m + 1], scale=1.0)

        # x_dbl partial projection -> cc_in (bf16)
        wxs = poolA.tile([128, 8, 96], BF16, tag="wxs")
        nc.sync.dma_start(out=wxs[:], in_=wx.ap().rearrange("(k p) m -> p k m", p=128))
        xdb = poolA.tile([96, L], F32, tag="xdb")
        for tn in range(2):
            ps96 = mm_ps.tile([96, TN], F32, tag="ps96", bufs=2)
            for k in range(8):
                nc.tensor.matmul(ps96[:], wxs[:, k, :],
                                 u3[:, k, tn * TN:(tn + 1) * TN],
                                 start=(k == 0), stop=(k == 7))
            nc.scalar.copy(out=xdb[:, tn * TN:(tn + 1) * TN], in_=ps96[:])
        nc.sync.dma_start(out=cc_in[:], in_=xdb[:])

        # z half of in_proj: emitted before the collective so its PE/ACT
        # work fills the AllReduce wait window
        for m in range(8):
            wi = wchunk.tile([128, 8, 128], BF16, tag="wi")
            nc.sync.dma_start(out=wi[:], in_=w_in_r[:, :, DH + m * 128: DH + (m + 1) * 128])
            for tn in range(2):
                ps = mm_ps.tile([128, TN], F32, tag="ps")
                for k in range(8):
                    nc.tensor.matmul(ps[:], wi[:, k, :],
                                     xts[:, k, tn * TN:(tn + 1) * TN],
                                     start=(k == 0), stop=(k == 7))
                nc.scalar.activation(out=szg[:, m, tn * TN:(tn + 1) * TN],
                                     in_=ps[:], func=AF.Silu,
                                     bias=bz[:, m:m + 1], scale=1.0)

        nc.gpsimd.collective_compute("AllReduce", OP.add, replica_groups=CC_GROUPS,
                                     ins=[cc_in[:]], outs=[cc_out[:]])
        sum32 = poolA.tile([96, L], F32, tag="sum32")
        nc.sync.dma_start(out=sum32[0:64, :],
                          in_=bass.AP(tensor=cc_out, offset=0, ap=[[L, 64], [1, L]]))
        nc.scalar.dma_start(out=sum32[64:96, :],
                            in_=bass.AP(tensor=cc_out, offset=64 * L, ap=[[L, 32], [1, L]]))
        dt16 = poolA.tile([64, L], BF16, tag="dt16")
        nc.vector.tensor_copy(dt16[:], sum32[0:64, :])
        bc16 = poolA.tile([32, L], BF16, tag="bc16")
        nc.vector.tensor_copy(bc16[:], sum32[64:96, :])
        nc.sync.dma_start(out=cc_sum[:], in_=bc16[:])

        # delta = softplus(dt @ Wdt.T + bdt), via exp then ln(1+x)
        wds = poolA.tile([64, NG, 128], BF16, tag="wds")
        nc.sync.dma_start(out=wds[:], in_=wdt.ap().rearrange("k (g p) -> k g p", p=128))
        for g in range(NG):
            for tn in range(2):
                psd = mm_ps.tile([128, TN], F32, tag="ps")
                nc.tensor.matmul(psd[:], wds[:, g, :], dt16[:, tn * TN:(tn + 1) * TN],
                                 start=True, stop=True)
                nc.scalar.activation(out=delta[:, g, tn * TN:(tn + 1) * TN],
                                     in_=psd[:], func=AF.Exp, bias=bd[:, g:g + 1], scale=1.0)
            if g % 2 == 1:
                nc.scalar.activation(out=delta[:, g - 1:g + 1, :], in_=delta[:, g - 1:g + 1, :],
                                     func=AF.Ln, bias=1.0, scale=1.0)
                nc.vector.tensor_tensor(out=du[:, g - 1:g + 1, :], in0=delta[:, g - 1:g + 1, :],
                                        in1=u3[:, g - 1:g + 1, :], op=OP.mult)
    poolA_cm.__exit__(None, None, None)

    # ---------------- Phase B: selective scan + out_proj ----------------
    with tc.tile_pool(name="wcp" + sfx, bufs=1) as wcp, \
         tc.tile_pool(name="bcpool" + sfx, bufs=6) as bcpool, \
         tc.tile_pool(name="sA" + sfx, bufs=2) as sA, \
         tc.tile_pool(name="dAkp" + sfx, bufs=2) as dAkp, \
         tc.tile_pool(name="sB" + sfx, bufs=3) as sB, \
         tc.tile_pool(name="sH" + sfx, bufs=4) as sH, \
         tc.tile_pool(name="ygbp" + sfx, bufs=2) as ygbp, \
         tc.tile_pool(name="oslp" + sfx, bufs=4) as oslp, \
         tc.tile_pool(name="ps_y" + sfx, bufs=4, space="PSUM") as psy_pool, \
         tc.tile_pool(name="op_ps" + sfx, bufs=4, space="PSUM") as op_ps:
        wco_r = wco.ap().rearrange("(k p) m -> p k m", p=128)
        outp_r = outp.ap().rearrange("b (m p) t -> b p m t", p=128)
        wc = []
        for mc in range(2):
            w_ = wcp.tile([128, 8, TN], BF16, tag=f"wc{mc}")
            nc.scalar.dma_start(out=w_[:], in_=wco_r[:, :, mc * TN:(mc + 1) * TN])
            wc.append(w_)

        idx = 0
        for blk in range(NB):
            g0 = blk * GB
            dAkeep = [None] * 4
            psy = [psy_pool.tile([128, TN], F32, tag="psy", name=f"psy{rep}_{blk}_{i}")
                   for i in range(4)]
            # tslice ts covers (group g0 + ts//2, columns (ts%2)*TN)
            for ts in range(4):
                g = g0 + ts // 2
                t0 = (ts % 2) * TN
                nc.tensor.matmul(psy[ts][:], ddg[:, g, :], u3[:, g, t0:t0 + TN],
                                 start=True, stop=False)
            for n in range(NST):
                bcp = bcpool.tile([128, 2, L], BF16, tag="bcp")
                nc.sync.dma_start(
                    out=bcp[:],
                    in_=bass.AP(tensor=cc_sum, offset=2 * n * L,
                                ap=[[0, 128], [L, 2], [1, L]]))
                brep = bass.AP(tensor=bcp.tensor, offset=bcp.offset,
                               ap=[[bcp.ap[0][0], 128], [0, GB], [1, L]])
                crep = bass.AP(tensor=bcp.tensor, offset=bcp.offset + L,
                               ap=[[bcp.ap[0][0], 128], [0, GB], [1, L]])
                if structured_a and n in (4, 5, 6, 7):
                    dA = dAkp.tile([128, GB, L], BF16, tag=f"dAk{n - 4}",
                                   name=f"dAk{rep}_{blk}_{n}")
                    dAkeep[n - 4] = dA
                else:
                    dA = sA.tile([128, GB, L], BF16, tag="dA")
                if structured_a and n in (9, 11, 13, 15):
                    src_dA = dAkeep[(n - 9) // 2]
                    sq_eng = nc.vector if n in (9, 13) else nc.gpsimd
                    sq_eng.tensor_tensor(out=dA[:], in0=src_dA[:], in1=src_dA[:],
                                         op=OP.mult)
                elif structured_a:
                    nc.scalar.activation(out=dA[:], in_=delta[:, g0:g0 + GB, :],
                                         func=AF.Exp, bias=0.0, scale=-float(n + 1))
                else:
                    for gg in range(GB):
                        nc.scalar.activation(out=dA[:, gg, :], in_=delta[:, g0 + gg, :],
                                             func=AF.Exp, bias=0.0,
                                             scale=At[:, g0 + gg, n:n + 1])
                dBu = sB.tile([128, GB, L], BF16, tag="dBu")
                nc.gpsimd.tensor_tensor(out=dBu[:], in0=du[:, g0:g0 + GB, :],
                                        in1=brep, op=OP.mult)
                h = sH.tile([128, GB, L], BF16, tag="h")
                for gg in range(GB):
                    nc.vector.tensor_tensor_scan(h[:, gg, :], dA[:, gg, :], dBu[:, gg, :],
                                                 0.0, OP.mult, OP.add)
                hc_eng = nc.gpsimd if (idx % 5 != 1) else nc.vector
                hc_eng.tensor_tensor(out=h[:], in0=h[:], in1=crep, op=OP.mult)
                for ts in range(4):
                    nc.tensor.matmul(psy[ts][:], idt[:],
                                     h[:, ts // 2, (ts % 2) * TN:(ts % 2) * TN + TN],
                                     start=False, stop=(n == NST - 1))
                idx += 1
            # gate with silu(z) and project this block's channels
            ygb = ygbp.tile([128, GB, L], BF16, tag="ygb")
            for ts in range(4):
                g = g0 + ts // 2
                t0 = (ts % 2) * TN
                nc.vector.tensor_tensor(out=ygb[:, ts // 2, t0:t0 + TN], in0=psy[ts][:],
                                          in1=szg[:, g, t0:t0 + TN], op=OP.mult)
            for mc in range(2):
                for mm in range(4):
                    m = mc * 4 + mm
                    for tn in range(2):
                        ps = op_ps.tile([128, TN], F32, tag="ps_o")
                        for kk in range(GB):
                            nc.tensor.matmul(ps[:], wc[mc][:, g0 + kk, mm * 128:(mm + 1) * 128],
                                             ygb[:, kk, tn * TN:(tn + 1) * TN],
                                             start=(kk == 0), stop=(kk == GB - 1))
                        osl = oslp.tile([128, TN], BF16, tag="osl")
                        if (mm + tn) % 2 == 0:
                            nc.scalar.copy(out=osl[:], in_=ps[:])
                        else:
                            nc.vector.tensor_copy(osl[:], ps[:])
                        nc.sync.dma_start(out=outp_r[blk, :, m, tn * TN:(tn + 1) * TN],
                                          in_=osl[:])
    bigp_cm.__exit__(None, None, None)


def _prep_core_inputs(inputs, b, d, h):
    pref = "f_" if d == 0 else "b_"
    g = lambda k: np.asarray(inputs[pref + k], dtype=np.float32)
    x = np.asarray(inputs["x"], dtype=np.float32)[b]
    if d == 1:
        x = x[::-1]
    sl = slice(h * DH, (h + 1) * DH)

    Win = g("Win")
    w_in = np.concatenate([Win[sl].T, Win[2048 + h * DH: 2048 + (h + 1) * DH].T], axis=1)
    convw = g("convw")[sl]
    convb_eff = g("convb")[sl] + g("bin")[sl] * convw.sum(-1)
    pg = lambda v: np.ascontiguousarray(v.reshape(NG, 128).T)
    pg3 = lambda v: np.ascontiguousarray(v.reshape(NG, 128, -1).transpose(1, 0, 2))
    A = -np.exp(g("Alog")[sl])
    proj_W = np.asarray(inputs["proj_W"], dtype=np.float32)
    Pd = proj_W[:, d * DM:(d + 1) * DM]
    wco = (Pd @ g("Wout"))[:, sl].T

    # x_dbl rows permuted: dt 0:64 unchanged, then B/C interleaved
    Wx = g("Wx")[:, sl]
    perm = list(range(64)) + [64 + 16 * (i % 2) + i // 2 for i in range(32)]
    Wx = Wx[perm]

    # diagonal weight matrices for the D-skip matmuls
    dskdiag = np.zeros((128, NG, 128), np.float32)
    dk = pg(g("Dsk")[sl])    # [128, NG]
    rng = np.arange(128)
    for gi in range(NG):
        dskdiag[rng, gi, rng] = dk[:, gi]

    import ml_dtypes
    bf = lambda v: np.ascontiguousarray(v).astype(ml_dtypes.bfloat16)
    return {
        "xT": bf(x.T),
        "w_in": bf(w_in),
        "wx": bf(Wx.T),
        "wdt": bf(g("Wdt")[sl].T),
        "wco": bf(wco),
        "convw": pg3(convw),
        "dskdiag": bf(dskdiag),
        "ident": bf(np.eye(128, dtype=np.float32)),
        "convb": pg(convb_eff),
        "binz": pg(g("bin")[2048 + h * DH: 2048 + (h + 1) * DH]),
        "bdt": pg(g("bdt")[sl]),
        "A_": pg3(A),
    }


def _check_structured_a(inputs):
    ar = np.log(np.arange(1, NST + 1, dtype=np.float32))
    for pref in ("f_", "b_"):
        Alog = np.asarray(inputs[pref + "Alog"], dtype=np.float32)
        if not np.allclose(Alog, np.broadcast_to(ar, Alog.shape), atol=1e-5):
            return False
    return True


_CACHE = {}


def _get_nc(structured_a, repeat=1):
    key = ("v2", structured_a, repeat)
    if key not in _CACHE:
        _CACHE[key] = _build(structured_a, repeat=repeat)
    return _CACHE[key]


def kernel(**inputs):
    from concourse.bass_utils import run_bass_kernel_spmd

    nc = _get_nc(_check_structured_a(inputs))
    in_maps = []
    for c in range(N_CORES):
        b, d, h = c >> 2, (c >> 1) & 1, c & 1
        in_maps.append(_prep_core_inputs(inputs, b, d, h))
    res = run_bass_kernel_spmd(nc, in_maps, list(range(N_CORES)))
    partials = [np.asarray(res.results[c]["outp"], dtype=np.float32).sum(axis=0)
                for c in range(N_CORES)]

    B = 2
    out = np.zeros((B, L, DM), np.float32)
    for b in range(B):
        for d in range(2):
            s = (partials[b * 4 + d * 2 + 0] + partials[b * 4 + d * 2 + 1]).T
            if d == 1:
                s = s[::-1]
            out[b] += s
    proj_W = np.asarray(inputs["proj_W"], dtype=np.float32)
    bias = (np.asarray(inputs["f_bout"], dtype=np.float32) @ proj_W[:, :DM].T
            + np.asarray(inputs["b_bout"], dtype=np.float32) @ proj_W[:, DM:].T
            + np.asarray(inputs["proj_b"], dtype=np.float32))
    return out + bias


# revision 21
# speedup vs baseline: 59.4903x; 1.1192x over previous
"""BiSSM (bidirectional Mamba block) Trainium2 kernel, v2.

Contract: kernel(**inputs) takes the FULL unsharded inputs of
nn_BiSSMBlock (see setup_inputs) and returns the full (2, 1024, 1024)
output.  Sharding: (batch 2) x (direction 2) x (d_inner half 2) over 8
cores; a pairwise bf16 AllReduce between half-cores for the x_dbl
projection.  Host folds Wout into proj_W per direction and sums/flips
partials.

vs the v1 kernel: in_proj runs in bf16 (weights shipped as bf16 from
host), the per-state y accumulation runs on the PE array (identity /
diag(Dsk) matmuls accumulating in PSUM, replacing the DVE/Pool add
tree), dA for states 9/11/13/15 is squared from kept lower states
instead of re-exp'd, the f32 AllReduce overlaps the z-half of in_proj
(AllGather and bf16 collectives are ~1 ms/dispatch in this runtime —
avoid), scans stay on DVE (GpSimd scans and scalar ops are rejected by
the walrus codegen; GpSimd also cannot read PSUM), and the output
projection emits bf16 block partials summed on host.

Self-contained: only needs the concourse/bass toolchain at
/opt/trn_rl_repo and 8 visible neuron cores.
"""
import sys
sys.path.insert(0, "/opt/trn_rl_repo")
import numpy as np

import concourse.bass as bass
import concourse.bacc as bacc
import concourse.mybir as mybir
import concourse.tile as tile

F32 = mybir.dt.float32
F32R = mybir.dt.float32r
BF16 = mybir.dt.bfloat16
OP = mybir.AluOpType
AF = mybir.ActivationFunctionType

L = 1024          # sequence length
DM = 1024         # d_model
DH = 1024         # d_inner half per core
NG = 8            # channel groups of 128 (DH/128)
NST = 16          # d_state
TN = 512          # matmul moving-dim tile
NB = 4            # phase-B blocks
GB = 2            # groups per block

N_CORES = 8
CC_GROUPS = [[0, 1], [2, 3], [4, 5], [6, 7]]


def _build(structured_a, repeat=1, n_cores=N_CORES):
    nc = bacc.Bacc("TRN2", target_bir_lowering=False, debug=False, num_devices=n_cores)

    xT = nc.declare_dram_parameter("xT", [DM, L], BF16, isOutput=False)
    w_in = nc.declare_dram_parameter("w_in", [DM, 2 * DH], BF16, isOutput=False)
    wx = nc.declare_dram_parameter("wx", [DH, 96], BF16, isOutput=False)
    wdt = nc.declare_dram_parameter("wdt", [64, DH], BF16, isOutput=False)
    wco = nc.declare_dram_parameter("wco", [DH, DM], BF16, isOutput=False)
    convw = nc.declare_dram_parameter("convw", [128, NG, 4], F32, isOutput=False)
    dskdiag = nc.declare_dram_parameter("dskdiag", [128, NG, 128], BF16, isOutput=False)
    ident = nc.declare_dram_parameter("ident", [128, 128], BF16, isOutput=False)
    convb = nc.declare_dram_parameter("convb", [128, NG], F32, isOutput=False)
    binz = nc.declare_dram_parameter("binz", [128, NG], F32, isOutput=False)
    bdt = nc.declare_dram_parameter("bdt", [128, NG], F32, isOutput=False)
    A_ = nc.declare_dram_parameter("A_", [128, NG, NST], F32, isOutput=False)
    outp = nc.declare_dram_parameter("outp", [NB, DM, L], BF16, isOutput=True)

    ccs = [(nc.dram_tensor(f"cc_in{r}", [96, L], F32),
            nc.dram_tensor(f"cc_out{r}", [96, L], F32),
            nc.dram_tensor(f"cc_sum{r}", [32, L], BF16)) for r in range(repeat)]

    with tile.TileContext(nc) as tc:
        consts_cm = tc.tile_pool(name="consts", bufs=1)
        consts = consts_cm.__enter__()
        cw = consts.tile([128, NG, 4], F32)
        nc.sync.dma_start(out=cw[:], in_=convw[:])
        ddg = consts.tile([128, NG, 128], BF16)
        nc.sync.dma_start(out=ddg[:], in_=dskdiag[:])
        idt = consts.tile([128, 128], BF16)
        nc.sync.dma_start(out=idt[:], in_=ident[:])
        cb = consts.tile([128, NG], F32)
        nc.sync.dma_start(out=cb[:], in_=convb[:])
        bz = consts.tile([128, NG], F32)
        nc.sync.dma_start(out=bz[:], in_=binz[:])
        bd = consts.tile([128, NG], F32)
        nc.sync.dma_start(out=bd[:], in_=bdt[:])
        At = consts.tile([128, NG, NST], F32)
        nc.sync.dma_start(out=At[:], in_=A_[:])

        for rep in range(repeat):
            cc_in, cc_out, cc_sum = ccs[rep]
            _emit_one(nc, tc, structured_a, rep,
                      xT, w_in, wx, wdt, wco, outp, cc_in, cc_out, cc_sum,
                      cw, ddg, idt, cb, bz, bd, At)

        consts_cm.__exit__(None, None, None)

    nc.compile()
    return nc


def _emit_one(nc, tc, structured_a, rep,
              xT, w_in, wx, wdt, wco, outp, cc_in, cc_out, cc_sum,
              cw, ddg, idt, cb, bz, bd, At):
    w_in_r = w_in.ap().rearrange("(k p) m -> p k m", p=128)
    sfx = f"_{rep}"

    bigp_cm = tc.tile_pool(name="bigp" + sfx, bufs=1)
    bigp = bigp_cm.__enter__()
    u3 = bigp.tile([128, NG, L], BF16, tag="u3")
    szg = bigp.tile([128, NG, L], BF16, tag="szg")
    delta = bigp.tile([128, NG, L], F32, tag="delta")
    du = bigp.tile([128, NG, L], BF16, tag="du")

    # ---------------- Phase A: projections + conv ----------------
    poolA_cm = tc.tile_pool(name="poolA" + sfx, bufs=1)
    poolA = poolA_cm.__enter__()
    with tc.tile_pool(name="wchunk" + sfx, bufs=2) as wchunk, \
         tc.tile_pool(name="scrp" + sfx, bufs=2) as scrp, \
         tc.tile_pool(name="mm_ps" + sfx, bufs=4, space="PSUM") as mm_ps:
        xts = poolA.tile([128, 8, L], BF16, tag="xts")
        for k in range(8):
            nc.sync.dma_start(
                out=xts[:, k, :],
                in_=xT.ap().rearrange("(k p) t -> p k t", p=128)[:, k, :])
        xin = poolA.tile([128, NG, L + 3], BF16, tag="xin")
        halo = bass.AP(tensor=xin.tensor, offset=xin.offset,
                       ap=[[xin.ap[0][0], 128], [L + 3, NG], [1, 3]])
        nc.vector.memset(halo, 0.0)

        # xin half of in_proj, then depthwise conv via diagonal matmuls
        for m in range(8):
            wi = wchunk.tile([128, 8, 128], BF16, tag="wi")
            nc.sync.dma_start(out=wi[:], in_=w_in_r[:, :, m * 128:(m + 1) * 128])
            for tn in range(2):
                ps = mm_ps.tile([128, TN], F32, tag="ps")
                for k in range(8):
                    nc.tensor.matmul(ps[:], wi[:, k, :],
                                     xts[:, k, tn * TN:(tn + 1) * TN],
                                     start=(k == 0), stop=(k == 7))
                nc.scalar.copy(out=xin[:, m, 3 + tn * TN: 3 + (tn + 1) * TN], in_=ps[:])
            cv_eng = nc.vector
            scr = scrp.tile([128, L], F32, tag="scr")
            cv_eng.tensor_scalar_mul(out=scr[:], in0=xin[:, m, 3:3 + L],
                                     scalar1=cw[:, m, 3:4])
            for k in range(3):
                cv_eng.scalar_tensor_tensor(
                    out=scr[:], in0=xin[:, m, k:k + L],
                    scalar=cw[:, m, k:k + 1], in1=scr[:],
                    op0=OP.mult, op1=OP.add)
            nc.scalar.activation(out=u3[:, m, :], in_=scr[:],
                                 func=AF.Silu, bias=cb[:, m:m + 1], scale=1.0)

        # x_dbl partial projection -> cc_in (bf16)
        wxs = poolA.tile([128, 8, 96], BF16, tag="wxs")
        nc.sync.dma_start(out=wxs[:], in_=wx.ap().rearrange("(k p) m -> p k m", p=128))
        xdb = poolA.tile([96, L], F32, tag="xdb")
        for tn in range(2):
            ps96 = mm_ps.tile([96, TN], F32, tag="ps96", bufs=2)
            for k in range(8):
                nc.tensor.matmul(ps96[:], wxs[:, k, :],
                                 u3[:, k, tn * TN:(tn + 1) * TN],
                                 start=(k == 0), stop=(k == 7))
            nc.scalar.copy(out=xdb[:, tn * TN:(tn + 1) * TN], in_=ps96[:])
        nc.sync.dma_start(out=cc_in[:], in_=xdb[:])

        # z half of in_proj: emitted before the collective so its PE/ACT
        # work fills the AllReduce wait window
        for m in range(8):
            wi = wchunk.tile([128, 8, 128], BF16, tag="wi")
            nc.sync.dma_start(out=wi[:], in_=w_in_r[:, :, DH + m * 128: DH + (m + 1) * 128])
            for tn in range(2):
                ps = mm_ps.tile([128, TN], F32, tag="ps")
                for k in range(8):
                    nc.tensor.matmul(ps[:], wi[:, k, :],
                                     xts[:, k, tn * TN:(tn + 1) * TN],
                                     start=(k == 0), stop=(k == 7))
                nc.scalar.activation(out=szg[:, m, tn * TN:(tn + 1) * TN],
                                     in_=ps[:], func=AF.Silu,
                                     bias=bz[:, m:m + 1], scale=1.0)

        nc.gpsimd.collective_compute("AllReduce", OP.add, replica_groups=CC_GROUPS,
                                     ins=[cc_in[:]], outs=[cc_out[:]])
        sum32 = poolA.tile([96, L], F32, tag="sum32")
        nc.sync.dma_start(out=sum32[0:64, :],
                          in_=bass.AP(tensor=cc_out, offset=0, ap=[[L, 64], [1, L]]))
        nc.scalar.dma_start(out=sum32[64:96, :],
                            in_=bass.AP(tensor=cc_out, offset=64 * L, ap=[[L, 32], [1, L]]))
        dt16 = poolA.tile([64, L], BF16, tag="dt16")
        nc.vector.tensor_copy(dt16[:], sum32[0:64, :])
        bc16 = poolA.tile([32, L], BF16, tag="bc16")
        nc.vector.tensor_copy(bc16[:], sum32[64:96, :])
        nc.sync.dma_start(out=cc_sum[:], in_=bc16[:])

        # delta = softplus(dt @ Wdt.T + bdt), via exp then ln(1+x)
        wds = poolA.tile([64, NG, 128], BF16, tag="wds")
        nc.sync.dma_start(out=wds[:], in_=wdt.ap().rearrange("k (g p) -> k g p", p=128))
        for g in range(NG):
            for tn in range(2):
                psd = mm_ps.tile([128, TN], F32, tag="ps")
                nc.tensor.matmul(psd[:], wds[:, g, :], dt16[:, tn * TN:(tn + 1) * TN],
                                 start=True, stop=True)
                nc.scalar.activation(out=delta[:, g, tn * TN:(tn + 1) * TN],
                                     in_=psd[:], func=AF.Exp, bias=bd[:, g:g + 1], scale=1.0)
            if g % 2 == 1:
                nc.scalar.activation(out=delta[:, g - 1:g + 1, :], in_=delta[:, g - 1:g + 1, :],
                                     func=AF.Ln, bias=1.0, scale=1.0)
                nc.vector.tensor_tensor(out=du[:, g - 1:g + 1, :], in0=delta[:, g - 1:g + 1, :],
                                        in1=u3[:, g - 1:g + 1, :], op=OP.mult)
    poolA_cm.__exit__(None, None, None)

    # ---------------- Phase B: selective scan + out_proj ----------------
    with tc.tile_pool(name="wcp" + sfx, bufs=1) as wcp, \
         tc.tile_pool(name="bcpool" + sfx, bufs=3) as bcpool, \
         tc.tile_pool(name="sA" + sfx, bufs=2) as sA, \
         tc.tile_pool(name="dAkp" + sfx, bufs=1) as dAkp, \
         tc.tile_pool(name="sB" + sfx, bufs=2) as sB, \
         tc.tile_pool(name="sH" + sfx, bufs=3) as sH, \
         tc.tile_pool(name="ygbp" + sfx, bufs=2) as ygbp, \
         tc.tile_pool(name="oslp" + sfx, bufs=4) as oslp, \
         tc.tile_pool(name="ps_y" + sfx, bufs=4, space="PSUM") as psy_pool, \
         tc.tile_pool(name="op_ps" + sfx, bufs=4, space="PSUM") as op_ps:
        wco_r = wco.ap().rearrange("(k p) m -> p k m", p=128)
        outp_r = outp.ap().rearrange("b (m p) t -> b p m t", p=128)
        wc = []
        for mc in range(2):
            w_ = wcp.tile([128, 8, TN], BF16, tag=f"wc{mc}")
            nc.scalar.dma_start(out=w_[:], in_=wco_r[:, :, mc * TN:(mc + 1) * TN])
            wc.append(w_)

        idx = 0
        for blk in range(NB):
            g0 = blk * GB
            dAkeep = {}
            psy = [psy_pool.tile([128, TN], F32, tag="psy", name=f"psy{rep}_{blk}_{i}")
                   for i in range(4)]
            # tslice ts covers (group g0 + ts//2, columns (ts%2)*TN)
            for ts in range(4):
                g = g0 + ts // 2
                t0 = (ts % 2) * TN
                nc.tensor.matmul(psy[ts][:], ddg[:, g, :], u3[:, g, t0:t0 + TN],
                                 start=True, stop=False)
            for np_ in range(NST // 2):
                n0 = 2 * np_  # states n0, n0+1 processed together
                bcp = bcpool.tile([128, 2, 2, L], BF16, tag="bcp")
                nc.sync.dma_start(
                    out=bcp[:],
                    in_=bass.AP(tensor=cc_sum, offset=2 * n0 * L,
                                ap=[[0, 128], [L, 4], [1, L]]))
                pstr = bcp.ap[0][0]
                # (state, group-rep, t) views of the B and C rows
                brep = bass.AP(tensor=bcp.tensor, offset=bcp.offset,
                               ap=[[pstr, 128], [2 * L, 2], [0, GB], [1, L]])
                crep = bass.AP(tensor=bcp.tensor, offset=bcp.offset + L,
                               ap=[[pstr, 128], [2 * L, 2], [0, GB], [1, L]])
                if structured_a and n0 in (4, 6):
                    dA = dAkp.tile([128, 2, GB, L], BF16, tag=f"dAk{n0}",
                                   name=f"dAk{rep}_{blk}_{n0}")
                    dAkeep[n0] = dA
                else:
                    dA = sA.tile([128, 2, GB, L], BF16, tag="dA")
                for s in range(2):
                    n = n0 + s
                    if structured_a and n in (9, 11, 13, 15):
                        kp = dAkeep[4 if n in (9, 11) else 6]
                        srcs = kp[:, (n - 9) // 2 % 2, :, :]
                        sq_eng = nc.vector if n in (9, 13) else nc.gpsimd
                        sq_eng.tensor_tensor(out=dA[:, s, :, :], in0=srcs, in1=srcs,
                                             op=OP.mult)
                    elif structured_a:
                        nc.scalar.activation(out=dA[:, s, :, :],
                                             in_=delta[:, g0:g0 + GB, :],
                                             func=AF.Exp, bias=0.0, scale=-float(n + 1))
                    else:
                        for gg in range(GB):
                            nc.scalar.activation(out=dA[:, s, gg, :],
                                                 in_=delta[:, g0 + gg, :],
                                                 func=AF.Exp, bias=0.0,
                                                 scale=At[:, g0 + gg, n:n + 1])
                dBu = sB.tile([128, 2, GB, L], BF16, tag="dBu")
                durep = bass.AP(tensor=du.tensor, offset=du.offset + g0 * L,
                                ap=[[du.ap[0][0], 128], [0, 2], [L, GB], [1, L]])
                nc.gpsimd.tensor_tensor(out=dBu[:], in0=durep, in1=brep, op=OP.mult)
                h = sH.tile([128, 2, GB, L], BF16, tag="h")
                # one flat scan over (2 states x GB groups); break the chain at
                # each interior segment start via dA[.., 0] = 0
                zpos = bass.AP(tensor=dA.tensor, offset=dA.offset + L,
                               ap=[[dA.ap[0][0], 128], [L, 2 * GB - 1], [1, 1]])
                nc.gpsimd.memset(zpos, 0.0)
                flat = lambda t: bass.AP(tensor=t.tensor, offset=t.offset,
                                         ap=[[t.ap[0][0], 128], [1, 2 * GB * L]])
                nc.vector.tensor_tensor_scan(flat(h), flat(dA), flat(dBu),
                                             0.0, OP.mult, OP.add)
                hc_eng = nc.gpsimd if (np_ % 3 == 2) else nc.vector
                hc_eng.tensor_tensor(out=h[:], in0=h[:], in1=crep, op=OP.mult)
                for s in range(2):
                    for ts in range(4):
                        nc.tensor.matmul(psy[ts][:], idt[:],
                                         h[:, s, ts // 2, (ts % 2) * TN:(ts % 2) * TN + TN],
                                         start=False, stop=(n0 + s == NST - 1))
                idx += 1
            # gate with silu(z) and project this block's channels
            ygb = ygbp.tile([128, GB, L], BF16, tag="ygb")
            for ts in range(4):
                g = g0 + ts // 2
                t0 = (ts % 2) * TN
                nc.vector.tensor_tensor(out=ygb[:, ts // 2, t0:t0 + TN], in0=psy[ts][:],
                                          in1=szg[:, g, t0:t0 + TN], op=OP.mult)
            for mc in range(2):
                for mm in range(4):
                    m = mc * 4 + mm
                    for tn in range(2):
                        ps = op_ps.tile([128, TN], F32, tag="ps_o")
                        for kk in range(GB):
                            nc.tensor.matmul(ps[:], wc[mc][:, g0 + kk, mm * 128:(mm + 1) * 128],
                                             ygb[:, kk, tn * TN:(tn + 1) * TN],
                                             start=(kk == 0), stop=(kk == GB - 1))
                        osl = oslp.tile([128, TN], BF16, tag="osl")
                        if (mm + tn) % 2 == 0:
                            nc.scalar.copy(out=osl[:], in_=ps[:])
                        else:
                            nc.vector.tensor_copy(osl[:], ps[:])
                        nc.sync.dma_start(out=outp_r[blk, :, m, tn * TN:(tn + 1) * TN],
                                          in_=osl[:])
    bigp_cm.__exit__(None, None, None)


def _prep_core_inputs(inputs, b, d, h):
    pref = "f_" if d == 0 else "b_"
    g = lambda k: np.asarray(inputs[pref + k], dtype=np.float32)
    x = np.asarray(inputs["x"], dtype=np.float32)[b]
    if d == 1:
        x = x[::-1]
    sl = slice(h * DH, (h + 1) * DH)

    Win = g("Win")
    w_in = np.concatenate([Win[sl].T, Win[2048 + h * DH: 2048 + (h + 1) * DH].T], axis=1)
    convw = g("convw")[sl]
    convb_eff = g("convb")[sl] + g("bin")[sl] * convw.sum(-1)
    pg = lambda v: np.ascontiguousarray(v.reshape(NG, 128).T)
    pg3 = lambda v: np.ascontiguousarray(v.reshape(NG, 128, -1).transpose(1, 0, 2))
    A = -np.exp(g("Alog")[sl])
    proj_W = np.asarray(inputs["proj_W"], dtype=np.float32)
    Pd = proj_W[:, d * DM:(d + 1) * DM]
    wco = (Pd @ g("Wout"))[:, sl].T

    # x_dbl rows permuted: dt 0:64 unchanged, then B/C interleaved
    Wx = g("Wx")[:, sl]
    perm = list(range(64)) + [64 + 16 * (i % 2) + i // 2 for i in range(32)]
    Wx = Wx[perm]

    # diagonal weight matrices for the D-skip matmuls
    dskdiag = np.zeros((128, NG, 128), np.float32)
    dk = pg(g("Dsk")[sl])    # [128, NG]
    rng = np.arange(128)
    for gi in range(NG):
        dskdiag[rng, gi, rng] = dk[:, gi]

    import ml_dtypes
    bf = lambda v: np.ascontiguousarray(v).astype(ml_dtypes.bfloat16)
    return {
        "xT": bf(x.T),
        "w_in": bf(w_in),
        "wx": bf(Wx.T),
        "wdt": bf(g("Wdt")[sl].T),
        "wco": bf(wco),
        "convw": pg3(convw),
        "dskdiag": bf(dskdiag),
        "ident": bf(np.eye(128, dtype=np.float32)),
        "convb": pg(convb_eff),
        "binz": pg(g("bin")[2048 + h * DH: 2048 + (h + 1) * DH]),
        "bdt": pg(g("bdt")[sl]),
        "A_": pg3(A),
    }


def _check_structured_a(inputs):
    ar = np.log(np.arange(1, NST + 1, dtype=np.float32))
    for pref in ("f_", "b_"):
        Alog = np.asarray(inputs[pref + "Alog"], dtype=np.float32)
        if not np.allclose(Alog, np.broadcast_to(ar, Alog.shape), atol=1e-5):
            return False
    return True


_CACHE = {}


def _get_nc(structured_a, repeat=1):
    key = ("v2", structured_a, repeat)
    if key not in _CACHE:
        _CACHE[key] = _build(structured_a, repeat=repeat)
    return _CACHE[key]


def kernel(**inputs):
    from concourse.bass_utils import run_bass_kernel_spmd

    nc = _get_nc(_check_structured_a(inputs))
    in_maps = []
    for c in range(N_CORES):
        b, d, h = c >> 2, (c >> 1) & 1, c & 1
        in_maps.append(_prep_core_inputs(inputs, b, d, h))
    res = run_bass_kernel_spmd(nc, in_maps, list(range(N_CORES)))
    partials = [np.asarray(res.results[c]["outp"], dtype=np.float32).sum(axis=0)
                for c in range(N_CORES)]

    B = 2
    out = np.zeros((B, L, DM), np.float32)
    for b in range(B):
        for d in range(2):
            s = (partials[b * 4 + d * 2 + 0] + partials[b * 4 + d * 2 + 1]).T
            if d == 1:
                s = s[::-1]
            out[b] += s
    proj_W = np.asarray(inputs["proj_W"], dtype=np.float32)
    bias = (np.asarray(inputs["f_bout"], dtype=np.float32) @ proj_W[:, :DM].T
            + np.asarray(inputs["b_bout"], dtype=np.float32) @ proj_W[:, DM:].T
            + np.asarray(inputs["proj_b"], dtype=np.float32))
    return out + bias
